# revision 12
# baseline (speedup 1.0000x reference)
"""Trainium2 Bass kernel for nn_MemLayer (retrieval_knn).

Math:  out[b,o] = -mean_d (x[b,d] - w[o,d])^2 + bias[o]
              =  s * (x' @ w'.T)[b,o]  -  ||x_b||^2/D  +  (bias[o] - ||w_o||^2/D)

  with x' = 16*x, w' = 4096*w in fp8e4m3 and s = 2/(D*16*4096). The GEMM term
  is ~1e-3 of the output magnitude, so the device only computes s*(x'@w'.T)
  (bf16 result); the exact rank-1 corrections are applied on the host in fp32.

Strategy:
  - Data-parallel shard x along batch across 8 NeuronCores (1024 rows each),
    replicate weights. No cross-core communication; gather on host.
  - Per core: fp8 GEMM [1024,1024] @ [1024,4096] with DoubleRow perf mode
    (contraction 256 per matmul, 256 matmuls of FD=512 -> 54.6us PE floor).
  - Schedule: nt (n-tile) outer so the 4MB weight stream trickles in at
    ~75GB/s; per nt, two half-groups of 4 m-tiles. Each half-group
    accumulates into ONE 4-bank PSUM tile [128, 4x512]; eviction is a single
    scale-only ACT into bf16 SBUF followed by a single 512KB DMA to DRAM on
    the same (Scalar) engine. The other 4 banks accumulate meanwhile.
  - Head: the first DMA pieces are small (128KB, >=1KB contiguous per
    partition) and ordered exactly in first-group consumption order across
    both HWDGE rings (Sync=wk, Scalar=xk) so the first real matmul starts
    ~2us after the rings open.
  - Tail: the final half-group is evicted in two 2-bank pieces with the DMAs
    issued on the otherwise-idle Sync ring, halving the post-compute drain.
  - No DVE work, no bias/x_sq tensors on device: corrections are host-side.
"""

import numpy as np
import ml_dtypes

B, D, O = 8192, 1024, 4096
NCORES = 8
BL = B // NCORES     # 1024 rows per core
P = 128
MT = BL // P         # 8 m-tiles
NTILE = 512          # one PSUM bank of fp32
NT = O // NTILE      # 8 n-tiles
GRP = 4              # m-tiles (PSUM banks) per eviction group

KD = D // (2 * P)    # 4 double-k-tiles (fp8 DoubleRow)
XSCALE = 16.0        # x -> fp8 pre-scale
WSCALE = 4096.0      # w -> fp8 pre-scale

_CACHE = {}


def _get_nc():
    key = "nc_v3"
    if key in _CACHE:
        return _CACHE[key]

    import concourse.bacc as bacc
    import concourse.tile as tile
    from concourse import mybir

    nc = bacc.Bacc("TRN2", target_bir_lowering=False)

    f32 = mybir.dt.float32
    bf16 = mybir.dt.bfloat16
    fp8 = mybir.dt.float8e4

    # x is half-major so every DMA piece is >=1KB-contiguous per partition
    xk_d = nc.dram_tensor("xk", [2, P, KD, 2, BL // 2], fp8,
                          kind="ExternalInput")
    wk_d = nc.dram_tensor("wk", [NT, P, KD, 2, NTILE], fp8, kind="ExternalInput")
    out_d = nc.dram_tensor("out", [P, NT * 2, GRP * NTILE], bf16,
                           kind="ExternalOutput")

    act_scale = float(2.0 / (D * XSCALE * WSCALE))
    DR = mybir.MatmulPerfMode.DoubleRow
    Ident = mybir.ActivationFunctionType.Identity

    with tile.TileContext(nc) as tc:
        with (
            tc.tile_pool(name="const", bufs=1) as cpool,
            tc.tile_pool(name="psum", bufs=2, space="PSUM") as ppool,
            tc.tile_pool(name="outp", bufs=4) as opool,
        ):
            xk_sb = cpool.tile([P, 2, KD, 2, BL // 2], fp8)
            wk_sb = cpool.tile([P, NT, KD, 2, NTILE], fp8)

            # Input pieces land in first-group consumption order on two
            # parallel HWDGE rings. Sync: weights; Scalar: x. The first x
            # piece covers only the first matmul's stationary tile so compute
            # is gated on the (bigger) first weight piece alone.
            nc.sync.dma_start(out=wk_sb[:, 0, 0], in_=wk_d[0, :, 0])
            nc.scalar.dma_start(out=xk_sb[:, 0, 0, :, 0:P],
                                in_=xk_d[0, :, 0, :, 0:P])
            nc.scalar.dma_start(out=xk_sb[:, 0, 0, :, P:],
                                in_=xk_d[0, :, 0, :, P:])
            for kc in range(1, KD):
                nc.sync.dma_start(out=wk_sb[:, 0, kc], in_=wk_d[0, :, kc])
                nc.scalar.dma_start(out=xk_sb[:, 0, kc], in_=xk_d[0, :, kc])
            nc.scalar.dma_start(out=xk_sb[:, 1], in_=xk_d[1])
            nc.sync.dma_start(out=wk_sb[:, 1], in_=wk_d[1])
            nc.sync.dma_start(out=wk_sb[:, 2], in_=wk_d[2])

            # Warmup: small matmuls on a zeroed tile keep the PE HAM activity
            # counter running while the input DMA head is in flight, so the
            # 1.2->2.4GHz unthrottle fires early. The PSUM bank is
            # overwritten by the first real accumulation group.
            zk = cpool.tile([P, 2, P], fp8)
            nc.gpsimd.memset(zk[:], 0.0)
            ps_warm = ppool.tile([P, GRP * NTILE], f32, tag="ps")
            for w in range(9):
                nc.tensor.matmul(
                    ps_warm[:, 0:P],
                    lhsT=zk[:],
                    rhs=zk[:],
                    start=True,
                    stop=True,
                    perf_mode=DR,
                )

            for nt in range(NT):
                if nt + 3 < NT:
                    nc.sync.dma_start(out=wk_sb[:, nt + 3], in_=wk_d[nt + 3])
                for half in range(2):
                    last = nt == NT - 1 and half == 1
                    ps = ppool.tile([P, GRP * NTILE], f32, tag="ps")
                    for kc in range(KD):
                        for j in range(GRP):
                            nc.tensor.matmul(
                                ps[:, j * NTILE:(j + 1) * NTILE],
                                lhsT=xk_sb[:, half, kc, :, j * P:(j + 1) * P],
                                rhs=wk_sb[:, nt, kc, :, :],
                                start=(kc == 0),
                                stop=(kc == KD - 1),
                                perf_mode=DR,
                            )
                    obs = opool.tile([P, GRP * NTILE], bf16, tag="obs")
                    if last:
                        # Final eviction: banks 0-1 on ACT, banks 2-3 on DVE
                        # in parallel; the two 256KB DMAs drain on separate
                        # HWDGE rings (Sync / Scalar).
                        hc = GRP * NTILE // 2
                        nc.scalar.activation(obs[:, 0:hc], ps[:, 0:hc], Ident,
                                             scale=act_scale)
                        nc.sync.dma_start(out=out_d[:, nt * 2 + half, 0:hc],
                                          in_=obs[:, 0:hc])
                        nc.vector.tensor_scalar_mul(obs[:, hc:], ps[:, hc:],
                                                    act_scale)
                        nc.scalar.dma_start(out=out_d[:, nt * 2 + half, hc:],
                                            in_=obs[:, hc:])
                    else:
                        nc.scalar.activation(obs[:], ps[:], Ident,
                                             scale=act_scale)
                        nc.scalar.dma_start(out=out_d[:, nt * 2 + half, :],
                                            in_=obs[:])

    nc.finalize()
    _CACHE[key] = nc
    return nc


def _prep_inputs(x, weights, bias):
    """Shard + lay out host inputs -> per-core in_maps (+ host corrections)."""
    x = np.asarray(x, dtype=np.float32)
    weights = np.asarray(weights, dtype=np.float32)
    bias = np.asarray(bias, dtype=np.float32)

    dt = ml_dtypes.float8_e4m3
    # k = kd*256 + i*128 + p
    wT = weights.T * np.float32(WSCALE)                       # [D, O]
    wk = np.ascontiguousarray(
        wT.reshape(KD, 2, P, NT, NTILE)
        .transpose(3, 2, 0, 1, 4)
        .astype(dt)
    )

    in_maps = []
    for c in range(NCORES):
        xs = x[c * BL:(c + 1) * BL]                            # [BL, D] fp32
        xT = xs.T                                              # [D, BL]
        # xk[h, p, kd, i, c] = x'[kd*256 + i*128 + p, h*512 + c]
        xk = np.ascontiguousarray(
            (xT.reshape(KD, 2, P, 2, BL // 2) * np.float32(XSCALE))
            .transpose(3, 2, 0, 1, 4)
            .astype(dt)
        )
        in_maps.append({"xk": xk, "wk": wk})

    # Host-side rank-1 corrections (exact fp32)
    w_sq = np.einsum("od,od->o", weights, weights)
    _CACHE["v"] = (bias - w_sq / np.float32(D)).astype(np.float32)     # [O]
    _CACHE["xsq"] = (-np.einsum("bd,bd->b", x, x) / np.float32(D)
                     ).astype(np.float32)                              # [B]
    return in_maps


def _gather(results):
    parts = []
    for c in range(NCORES):
        o = np.asarray(results[c]["out"])            # [P, NT*2, GRP*NTILE] bf16
        o = o.reshape(P, NT, 2, GRP, NTILE)
        # b_local = (half*GRP + j)*P + p ; o_col = nt*NTILE + col
        o = o.transpose(2, 3, 0, 1, 4).reshape(BL, O)
        parts.append(o)
    full = np.concatenate(parts, axis=0).astype(np.float32)
    full += _CACHE["xsq"][:, None]
    full += _CACHE["v"][None, :]
    return np.ascontiguousarray(full)


def _run(in_maps, **kwargs):
    from concourse.bass_utils import run_bass_kernel_spmd

    nc = _get_nc()
    return run_bass_kernel_spmd(nc, in_maps, core_ids=list(range(NCORES)), **kwargs)


def kernel(x, weights, bias):
    in_maps = _prep_inputs(x, weights, bias)
    res = _run(in_maps)
    return _gather(res.results)


# revision 13
# speedup vs baseline: 1.0119x; 1.0119x over previous
"""Trainium2 Bass kernel for nn_MemLayer (retrieval_knn).

Math:  out[b,o] = -mean_d (x[b,d] - w[o,d])^2 + bias[o]
              =  s * (x' @ w'.T)[b,o]  -  ||x_b||^2/D  +  (bias[o] - ||w_o||^2/D)

  with x' = 16*x, w' = 4096*w in fp8e4m3 and s = 2/(D*16*4096). The GEMM term
  is ~1e-3 of the output magnitude, so the device only computes s*(x'@w'.T)
  (bf16 result); the exact rank-1 corrections are applied on the host in fp32.

Strategy:
  - Data-parallel shard x along batch across 8 NeuronCores (1024 rows each),
    replicate weights. No cross-core communication; gather on host.
  - Per core: fp8 GEMM [1024,1024] @ [1024,4096] with DoubleRow perf mode
    (contraction 256 per matmul, 256 matmuls of FD=512 -> 54.6us PE floor).
  - Schedule: nt (n-tile) outer so the 4MB weight stream trickles in at
    ~75GB/s; per nt, two half-groups of 4 m-tiles. Each half-group
    accumulates into ONE 4-bank PSUM tile [128, 4x512]; eviction is a single
    scale-only ACT into bf16 SBUF followed by a single 512KB DMA to DRAM on
    the same (Scalar) engine. The other 4 banks accumulate meanwhile.
  - Head: the first DMA pieces are small (128KB, >=1KB contiguous per
    partition) and ordered exactly in first-group consumption order across
    both HWDGE rings (Sync=wk, Scalar=xk) so the first real matmul starts
    ~2us after the rings open.
  - Tail: the final half-group is evicted in two 2-bank pieces with the DMAs
    issued on the otherwise-idle Sync ring, halving the post-compute drain.
  - No DVE work, no bias/x_sq tensors on device: corrections are host-side.
"""

import numpy as np
import ml_dtypes

B, D, O = 8192, 1024, 4096
NCORES = 8
BL = B // NCORES     # 1024 rows per core
P = 128
MT = BL // P         # 8 m-tiles
NTILE = 512          # one PSUM bank of fp32
NT = O // NTILE      # 8 n-tiles
GRP = 4              # m-tiles (PSUM banks) per eviction group

KD = D // (2 * P)    # 4 double-k-tiles (fp8 DoubleRow)
XSCALE = 16.0        # x -> fp8 pre-scale
WSCALE = 4096.0      # w -> fp8 pre-scale

_CACHE = {}


def _get_nc():
    key = "nc_v3"
    if key in _CACHE:
        return _CACHE[key]

    import concourse.bacc as bacc
    import concourse.tile as tile
    from concourse import mybir

    nc = bacc.Bacc("TRN2", target_bir_lowering=False)

    f32 = mybir.dt.float32
    bf16 = mybir.dt.bfloat16
    fp8 = mybir.dt.float8e4

    # x is half-major so every DMA piece is >=1KB-contiguous per partition
    xk_d = nc.dram_tensor("xk", [2, P, KD, 2, BL // 2], fp8,
                          kind="ExternalInput")
    wk_d = nc.dram_tensor("wk", [NT, P, KD, 2, NTILE], fp8, kind="ExternalInput")
    out_d = nc.dram_tensor("out", [P, NT * 2, GRP * NTILE], bf16,
                           kind="ExternalOutput")

    act_scale = float(2.0 / (D * XSCALE * WSCALE))
    DR = mybir.MatmulPerfMode.DoubleRow
    Ident = mybir.ActivationFunctionType.Identity

    with tile.TileContext(nc) as tc:
        with (
            tc.tile_pool(name="const", bufs=1) as cpool,
            tc.tile_pool(name="psum", bufs=2, space="PSUM") as ppool,
            tc.tile_pool(name="outp", bufs=4) as opool,
        ):
            xk_sb = cpool.tile([P, 2, KD, 2, BL // 2], fp8)
            wk_sb = cpool.tile([P, NT, KD, 2, NTILE], fp8)

            # Input pieces land in first-group consumption order on two
            # parallel HWDGE rings. Sync: weights; Scalar: x. The first x
            # piece covers only the first matmul's stationary tile so compute
            # is gated on the (bigger) first weight piece alone.
            zk = cpool.tile([P, 2, P], fp8)
            nc.gpsimd.memset(zk[:], 0.0)
            nc.gpsimd.dma_start(out=xk_sb[:, 1], in_=xk_d[1])

            nc.sync.dma_start(out=wk_sb[:, 0, 0], in_=wk_d[0, :, 0])
            nc.scalar.dma_start(out=xk_sb[:, 0, 0, :, 0:P],
                                in_=xk_d[0, :, 0, :, 0:P])
            nc.scalar.dma_start(out=xk_sb[:, 0, 0, :, P:],
                                in_=xk_d[0, :, 0, :, P:])
            for kc in range(1, KD):
                nc.sync.dma_start(out=wk_sb[:, 0, kc], in_=wk_d[0, :, kc])
                nc.scalar.dma_start(out=xk_sb[:, 0, kc], in_=xk_d[0, :, kc])
            nc.sync.dma_start(out=wk_sb[:, 1], in_=wk_d[1])
            nc.sync.dma_start(out=wk_sb[:, 2], in_=wk_d[2])

            # Warmup: small matmuls on a zeroed tile keep the PE HAM activity
            # counter running while the input DMA head is in flight, so the
            # 1.2->2.4GHz unthrottle fires early. The PSUM bank is
            # overwritten by the first real accumulation group.
            ps_warm = ppool.tile([P, GRP * NTILE], f32, tag="ps")
            for w in range(7):
                nc.tensor.matmul(
                    ps_warm[:, 0:P],
                    lhsT=zk[:],
                    rhs=zk[:],
                    start=True,
                    stop=True,
                    perf_mode=DR,
                )

            for nt in range(NT):
                if nt + 3 < NT:
                    nc.sync.dma_start(out=wk_sb[:, nt + 3], in_=wk_d[nt + 3])
                for half in range(2):
                    last = nt == NT - 1 and half == 1
                    ps = ppool.tile([P, GRP * NTILE], f32, tag="ps")
                    for kc in range(KD):
                        for j in range(GRP):
                            nc.tensor.matmul(
                                ps[:, j * NTILE:(j + 1) * NTILE],
                                lhsT=xk_sb[:, half, kc, :, j * P:(j + 1) * P],
                                rhs=wk_sb[:, nt, kc, :, :],
                                start=(kc == 0),
                                stop=(kc == KD - 1),
                                perf_mode=DR,
                            )
                    obs = opool.tile([P, GRP * NTILE], bf16, tag="obs")
                    if last:
                        # Final eviction: banks 0-1 on ACT, banks 2-3 on DVE
                        # in parallel; the two 256KB DMAs drain on separate
                        # HWDGE rings (Sync / Scalar).
                        hc = GRP * NTILE // 2
                        nc.scalar.activation(obs[:, 0:hc], ps[:, 0:hc], Ident,
                                             scale=act_scale)
                        nc.sync.dma_start(out=out_d[:, nt * 2 + half, 0:hc],
                                          in_=obs[:, 0:hc])
                        nc.vector.tensor_scalar_mul(obs[:, hc:], ps[:, hc:],
                                                    act_scale)
                        nc.scalar.dma_start(out=out_d[:, nt * 2 + half, hc:],
                                            in_=obs[:, hc:])
                    else:
                        nc.scalar.activation(obs[:], ps[:], Ident,
                                             scale=act_scale)
                        nc.scalar.dma_start(out=out_d[:, nt * 2 + half, :],
                                            in_=obs[:])

    nc.finalize()
    _CACHE[key] = nc
    return nc


def _prep_inputs(x, weights, bias):
    """Shard + lay out host inputs -> per-core in_maps (+ host corrections)."""
    x = np.asarray(x, dtype=np.float32)
    weights = np.asarray(weights, dtype=np.float32)
    bias = np.asarray(bias, dtype=np.float32)

    dt = ml_dtypes.float8_e4m3
    # k = kd*256 + i*128 + p
    wT = weights.T * np.float32(WSCALE)                       # [D, O]
    wk = np.ascontiguousarray(
        wT.reshape(KD, 2, P, NT, NTILE)
        .transpose(3, 2, 0, 1, 4)
        .astype(dt)
    )

    in_maps = []
    for c in range(NCORES):
        xs = x[c * BL:(c + 1) * BL]                            # [BL, D] fp32
        xT = xs.T                                              # [D, BL]
        # xk[h, p, kd, i, c] = x'[kd*256 + i*128 + p, h*512 + c]
        xk = np.ascontiguousarray(
            (xT.reshape(KD, 2, P, 2, BL // 2) * np.float32(XSCALE))
            .transpose(3, 2, 0, 1, 4)
            .astype(dt)
        )
        in_maps.append({"xk": xk, "wk": wk})

    # Host-side rank-1 corrections (exact fp32)
    w_sq = np.einsum("od,od->o", weights, weights)
    _CACHE["v"] = (bias - w_sq / np.float32(D)).astype(np.float32)     # [O]
    _CACHE["xsq"] = (-np.einsum("bd,bd->b", x, x) / np.float32(D)
                     ).astype(np.float32)                              # [B]
    return in_maps


def _gather(results):
    parts = []
    for c in range(NCORES):
        o = np.asarray(results[c]["out"])            # [P, NT*2, GRP*NTILE] bf16
        o = o.reshape(P, NT, 2, GRP, NTILE)
        # b_local = (half*GRP + j)*P + p ; o_col = nt*NTILE + col
        o = o.transpose(2, 3, 0, 1, 4).reshape(BL, O)
        parts.append(o)
    full = np.concatenate(parts, axis=0).astype(np.float32)
    full += _CACHE["xsq"][:, None]
    full += _CACHE["v"][None, :]
    return np.ascontiguousarray(full)


def _run(in_maps, **kwargs):
    from concourse.bass_utils import run_bass_kernel_spmd

    nc = _get_nc()
    return run_bass_kernel_spmd(nc, in_maps, core_ids=list(range(NCORES)), **kwargs)


def kernel(x, weights, bias):
    in_maps = _prep_inputs(x, weights, bias)
    res = _run(in_maps)
    return _gather(res.results)
